# revision 1
# baseline (speedup 1.0000x reference)
"""Trainium2 Bass kernel for nn_MultiHeadAttention (B=4, S=2048, D=512, H=8, DH=64).

Sharding: 8 cores = 4 batches x 2 query-halves. Each core computes full
attention for all 8 heads over its 1024 query rows (K/V projections are
duplicated within a batch pair). The output is a pure concatenation.

Per-core pipeline (bf16 datapath, fp32 PSUM accumulation):
  1. Inputs/weights are pre-cast to bf16 on the host; X^T (feature-major)
     loads straight from HBM via DMA xbar transpose.
  2. Project: Q^T, K^T feature-major ([512, S]); V natural ([S, 512]) with an
     extra all-ones column appended per head (65-col layout).
  3. Attention per head, per 128-row k-block:
       S^T[k, q] = K^T_h(stationary) @ Q^T_h   (contraction = d_head 64)
       P^T = exp(S^T / 8)                       (ScalarE, PSUM -> SBUF bf16)
       z^T[65, q] += [V_h | 1](stationary) @ P^T  (row 64 = softmax denom)
  4. Normalize: reciprocal of row 64, broadcast, multiply -> Z^T.
  5. Output projection from Z^T + bias in fp32, DMA out.
"""

import os
import sys

import numpy as np

sys.path.insert(0, "/opt/trn_rl_repo")

import ml_dtypes
import concourse.bacc as bacc
import concourse.bass as bass
import concourse.mybir as mybir
import concourse.tile as tile
from concourse import bass_utils

F32 = mybir.dt.float32
BF16 = mybir.dt.bfloat16

B, S, D, H, DH = 4, 2048, 512, 8, 64
SQ = S // 2          # query rows per core
NKB = S // 128       # 16 k-blocks
NFT = D // 128       # 4 feature tiles
NQB = SQ // 128      # 8 query blocks
N_CORES = 8

Exp = mybir.ActivationFunctionType.Exp
Identity = mybir.ActivationFunctionType.Identity


def build_program(dbg=False):
    nc = bacc.Bacc("TRN2", target_bir_lowering=False, debug=False)
    dbg_out = {}
    if dbg:
        for nm, shp, dt in [("KT0", [128, S], BF16), ("QT0", [128, SQ], BF16),
                            ("VA0", [128, 520], BF16), ("PA00", [128, SQ], BF16),
                            ("ZA0", [65, SQ], F32), ("ZT0", [128, SQ], BF16),
                            ("XKT0", [128, S], BF16)]:
            dbg_out[nm] = nc.dram_tensor(nm, shp, dt, kind="ExternalOutput").ap()

    xq = nc.dram_tensor("XQ", [SQ, D], BF16, kind="ExternalInput").ap()
    xk = nc.dram_tensor("XK", [S, D], BF16, kind="ExternalInput").ap()
    xv = nc.dram_tensor("XV", [S, D], BF16, kind="ExternalInput").ap()
    wq = nc.dram_tensor("Wq", [D, D], BF16, kind="ExternalInput").ap()
    wk = nc.dram_tensor("Wk", [D, D], BF16, kind="ExternalInput").ap()
    wv = nc.dram_tensor("Wv", [D, D], BF16, kind="ExternalInput").ap()
    wo = nc.dram_tensor("Wo", [D, D], BF16, kind="ExternalInput").ap()
    bq = nc.dram_tensor("bq", [D, 1], F32, kind="ExternalInput").ap()
    bk = nc.dram_tensor("bk", [D, 1], F32, kind="ExternalInput").ap()
    bv = nc.dram_tensor("bv", [1, D], F32, kind="ExternalInput").ap()
    bo = nc.dram_tensor("bo", [1, D], F32, kind="ExternalInput").ap()
    out = nc.dram_tensor("OUT", [SQ, D], F32, kind="ExternalOutput").ap()

    from contextlib import ExitStack

    with tile.TileContext(nc) as tc, ExitStack() as ctx:
        const = ctx.enter_context(tc.tile_pool(name="const", bufs=1))
        xt_pool = ctx.enter_context(tc.tile_pool(name="xt", bufs=1))
        w_pool = ctx.enter_context(tc.tile_pool(name="w", bufs=1))
        kt_pool = ctx.enter_context(tc.tile_pool(name="kt", bufs=1))
        qt_pool = ctx.enter_context(tc.tile_pool(name="qt", bufs=1))
        v_pool = ctx.enter_context(tc.tile_pool(name="v", bufs=1))
        p_pool = ctx.enter_context(tc.tile_pool(name="p", bufs=23))
        zt_pool = ctx.enter_context(tc.tile_pool(name="zt", bufs=1))
        nrm_pool = ctx.enter_context(tc.tile_pool(name="nrm", bufs=3))
        out_pool = ctx.enter_context(tc.tile_pool(name="outp", bufs=2))

        # One PSUM pool, 4 tags x 2 banks = all 8 banks.  Prefix (projection)
        # and epilogue tiles rotate through the same tags that attention uses
        # for sA/sB/zA/zB.
        ps = ctx.enter_context(tc.tile_pool(name="ps", bufs=1, space="PSUM"))
        ps_ctr = [0, 0]

        def ps_tile(shape, tags, name):
            i = 0 if tags == "ab" else 1
            tag = ("a", "b", "c", "d")[2 * i + ps_ctr[i] % 2]
            ps_ctr[i] += 1
            return ps.tile(shape, F32, tag=tag, name=name, padded_shape=[128, SQ])

        # ---- weights: one DMA per tensor, sliced into 4 contraction chunks ----
        def load_w(wdram, name, eng):
            big = w_pool.tile([128, NFT * D], BF16, tag=f"w{name}", name=f"w{name}")
            eng.dma_start(
                big[:].rearrange("p (g c) -> p g c", g=NFT),
                wdram.rearrange("(g p) c -> p g c", p=128),
            )
            return [big[:, D * mc:D * (mc + 1)] for mc in range(NFT)]

        # ---- X^T via DMA xbar transpose ----
        def load_xt(xdram, nrows, name, engs):
            tiles = []
            for ft in range(NFT):
                t = xt_pool.tile([128, nrows], BF16, tag=f"xt{name}{ft}",
                                 name=f"xt{name}{ft}", padded_shape=[128, S])
                engs[ft % len(engs)].dma_start(
                    t[:], xdram[:, 128 * ft:128 * (ft + 1)], transpose=True
                )
                tiles.append(t)
            return tiles

        # ---- DMA loads, ordered by first use ----
        wk_t = load_w(wk, "k", nc.sync)
        xkt = load_xt(xk, S, "k", [nc.sync])
        bq_all = const.tile([128, NFT], F32, tag="bqa")
        nc.sync.dma_start(
            bq_all[:].rearrange("p (g o) -> p g o", g=NFT),
            bq.rearrange("(g p) o -> p g o", p=128),
        )
        bk_all = const.tile([128, NFT], F32, tag="bka")
        nc.sync.dma_start(
            bk_all[:].rearrange("p (g o) -> p g o", g=NFT),
            bk.rearrange("(g p) o -> p g o", p=128),
        )
        bq_t = [bq_all[:, ft:ft + 1] for ft in range(NFT)]
        bk_t = [bk_all[:, ft:ft + 1] for ft in range(NFT)]
        bv_row = const.tile([1, D], F32, tag="bvr")
        nc.sync.dma_start(bv_row[:], bv[:])
        bv_bc = const.tile([128, D], F32, tag="bvb")
        nc.gpsimd.partition_broadcast(bv_bc[:], bv_row[:], channels=128)
        bo_row = const.tile([1, D], F32, tag="bor")
        nc.sync.dma_start(bo_row[:], bo[:])
        bo_bc = const.tile([128, D], F32, tag="bob")
        nc.gpsimd.partition_broadcast(bo_bc[:], bo_row[:], channels=128)

        wq_t = load_w(wq, "q", nc.sync)
        xqt = load_xt(xq, SQ, "q", [nc.sync])
        wv_t = load_w(wv, "v", nc.sync)
        xvt = load_xt(xv, S, "v", [nc.sync])
        wo_t = load_w(wo, "o", nc.sync)

        k_t = [kt_pool.tile([128, S], BF16, tag=f"kt{ft}", name=f"kt{ft}")
               for ft in range(NFT)]
        q_t = [qt_pool.tile([128, SQ], BF16, tag=f"qt{ft}", name=f"qt{ft}")
               for ft in range(NFT)]

        def proj_k_chunk(ft, sc):
            pj = ps_tile([128, 1024], "cd", f"pjk{ft}{sc}")
            for h2 in range(2):
                for mc in range(NFT):
                    nc.tensor.matmul(
                        pj[:, 512 * h2:512 * (h2 + 1)],
                        wk_t[mc][:, 128 * ft:128 * (ft + 1)],
                        xkt[mc][:, 1024 * sc + 512 * h2:1024 * sc + 512 * (h2 + 1)],
                        start=(mc == 0),
                        stop=(mc == NFT - 1),
                    )
            nc.any.tensor_scalar_add(
                k_t[ft][:, 1024 * sc:1024 * (sc + 1)], pj[:], bk_t[ft][:],
            )

        def proj_q_chunk(ft):
            pj = ps_tile([128, 1024], "cd", f"pjq{ft}")
            for h2 in range(2):
                for mc in range(NFT):
                    nc.tensor.matmul(
                        pj[:, 512 * h2:512 * (h2 + 1)],
                        wq_t[mc][:, 128 * ft:128 * (ft + 1)],
                        xqt[mc][:, 512 * h2:512 * (h2 + 1)],
                        start=(mc == 0),
                        stop=(mc == NFT - 1),
                    )
            nc.any.tensor_scalar_add(q_t[ft][:], pj[:], bq_t[ft][:])

        def proj_kq(ft):
            proj_k_chunk(ft, 0)
            proj_q_chunk(ft)
            proj_k_chunk(ft, 1)

        # warm the ScalarE Exp table before the critical path (the first
        # ACTIVATE otherwise pays the ~2.7us ACT_TABLE_LOAD inline)
        warm = nrm_pool.tile([1, 8], F32, tag="warm")
        nc.gpsimd.memset(warm[:], 0.0)
        warm2 = nrm_pool.tile([1, 8], F32, tag="warm2")
        nc.scalar.activation(warm2[:], warm[:], Exp, scale=0.125)

        # ---- slot-scheduled emission ----------------------------------
        # PE is the binding engine; emit its work as one interleaved stream:
        #  - S + exp for (pair, kb) runs in slot (pair, kb)
        #  - V projections ride in pair-0 slots (PSUM c/d tags)
        #  - each pair's z-accumulation is deferred while c/d is busy, then
        #    drains two-groups-per-slot once its zA/zB tiles pin c/d
        #  - K/Q projections for pair p+1 slot into the c/d window between
        #    norm(p-1) and z(p) pinning
        proj_k_chunk(0, 0)
        proj_q_chunk(0)

        VW = H * (DH + 1)  # 520: per head 64 value cols + 1 ones col
        v_aug = [v_pool.tile([128, VW], BF16, tag=f"v{kb}", name=f"v{kb}")
                 for kb in range(NKB)]

        def v_group(kb):
            nc.gpsimd.memset(
                v_aug[kb][:].rearrange("p (h c) -> p h c", h=H)[:, :, DH:DH + 1],
                1.0,
            )
            pj = ps_tile([128, 512], "cd", f"pjv{kb}")
            for mc in range(NFT):
                nc.tensor.matmul(
                    pj[:],
                    xvt[mc][:, 128 * kb:128 * (kb + 1)],
                    wv_t[mc][:],
                    start=(mc == 0),
                    stop=(mc == NFT - 1),
                )
            nc.any.tensor_add(
                v_aug[kb][:].rearrange("p (h c) -> p h c", h=H)[:, :, 0:DH],
                pj[:].rearrange("p (h c) -> p h c", h=H),
                bv_bc[:].rearrange("p (h c) -> p h c", h=H),
            )

        z_t = [zt_pool.tile([128, SQ], BF16, tag=f"zt{p}", name=f"zt{p}")
               for p in range(NFT)]
        p_slabs = {}
        z_tiles = {}

        def s_exp(pair, kb):
            sA = ps.tile([128, SQ], F32, tag="a", name=f"sA{pair}_{kb}")
            sB = ps.tile([128, SQ], F32, tag="b", name=f"sB{pair}_{kb}")
            for qc in range(SQ // 512):
                qs = slice(512 * qc, 512 * (qc + 1))
                nc.tensor.matmul(
                    sA[:, qs],
                    k_t[pair][0:DH, 128 * kb:128 * (kb + 1)],
                    q_t[pair][0:DH, qs],
                    start=True, stop=True,
                    tile_position=(0, 0),
                )
                nc.tensor.matmul(
                    sB[:, qs],
                    k_t[pair][DH:128, 128 * kb:128 * (kb + 1)],
                    q_t[pair][DH:128, qs],
                    start=True, stop=True,
                    tile_position=(64, 0),
                )
            pA = p_pool.tile([128, SQ], BF16, tag="p", name=f"pA{pair}_{kb}")
            nc.scalar.activation(pA[:], sA[:], Exp, scale=0.125)
            pB = p_pool.tile([128, SQ], BF16, tag="p", name=f"pB{pair}_{kb}")
            nc.scalar.activation(pB[:], sB[:], Exp, scale=0.125)
            if dbg and pair == 0 and kb == 0:
                nc.sync.dma_start(dbg_out["PA00"][:], pA[:])
            p_slabs[(pair, kb)] = (pA, pB)

        def z_alloc(pair):
            zA = ps.tile([DH + 1, SQ], F32, tag="c", name=f"zA{pair}",
                         padded_shape=[128, SQ])
            zB = ps.tile([DH + 1, SQ], F32, tag="d", name=f"zB{pair}",
                         padded_shape=[128, SQ])
            z_tiles[pair] = (zA, zB)

        def z_group(pair, kb):
            zA, zB = z_tiles[pair]
            pA, pB = p_slabs.pop((pair, kb))
            hA, hB = 2 * pair, 2 * pair + 1
            for qc in range(SQ // 512):
                qs = slice(512 * qc, 512 * (qc + 1))
                nc.tensor.matmul(
                    zA[:, qs],
                    v_aug[kb][:, 65 * hA:65 * hA + 65],
                    pA[:, qs],
                    start=(kb == 0), stop=(kb == NKB - 1),
                    skip_group_check=True,
                )
                nc.tensor.matmul(
                    zB[:, qs],
                    v_aug[kb][:, 65 * hB:65 * hB + 65],
                    pB[:, qs],
                    start=(kb == 0), stop=(kb == NKB - 1),
                    skip_group_check=True,
                )

        def norm(pair):
            zA, zB = z_tiles.pop(pair)
            if dbg and pair == 0:
                zdump = out_pool.tile([65, SQ], F32, tag="zdump")
                nc.vector.tensor_copy(zdump[:], zA[:])
                nc.sync.dma_start(dbg_out["ZA0"][:], zdump[:])
            for z_ps, half in ((zA, 0), (zB, 1)):
                # custom-DVE recip mis-reads PSUM: stage the row via ScalarE
                rowc = nrm_pool.tile([1, SQ], F32, tag="rowc")
                nc.scalar.activation(rowc[:], z_ps[DH:DH + 1, :],
                                     mybir.ActivationFunctionType.Copy)
                recip = nrm_pool.tile([1, SQ], F32, tag="recip")
                nc.vector.reciprocal_approx_fast(recip[:], rowc[:])
                rbc = nrm_pool.tile([DH, SQ], F32, tag="rbc")
                nc.gpsimd.partition_broadcast(rbc[:], recip[:], channels=DH)
                nc.vector.tensor_mul(
                    z_t[pair][64 * half:64 * half + 64, :], z_ps[0:DH, :], rbc[:]
                )

        # slot schedule: slot (p, kb) -> extra emissions after S+exp
        feeder = [lambda: proj_k_chunk(0, 1),
                  lambda: proj_k_chunk(1, 0),
                  lambda: proj_q_chunk(1),
                  lambda: proj_k_chunk(1, 1)]
        feeder += [(lambda k: (lambda: v_group(k)))(kb) for kb in range(NKB)]
        fi = [0]

        def feed(n):
            for _ in range(n):
                if fi[0] < len(feeder):
                    feeder[fi[0]]()
                    fi[0] += 1

        for pair in range(NFT):
            for kb in range(NKB):
                s_exp(pair, kb)
                if pair == 0:
                    feed(2 if kb < 4 else 1)
                    if kb == NKB - 1:
                        feed(len(feeder))
                elif pair == 1:
                    if kb == 0:
                        z_alloc(0)
                    if kb < 8:
                        z_group(0, 2 * kb)
                        z_group(0, 2 * kb + 1)
                    elif kb == 8:
                        norm(0)
                    elif kb == 9:
                        proj_kq(2)
                    elif kb == 12:
                        z_alloc(1)
                    if kb >= 12:
                        z_group(1, 2 * (kb - 12))
                        z_group(1, 2 * (kb - 12) + 1)
                elif pair == 2:
                    if kb < 4:
                        z_group(1, 8 + 2 * kb)
                        z_group(1, 8 + 2 * kb + 1)
                    elif kb == 4:
                        norm(1)
                    elif kb == 5:
                        proj_kq(3)
                    elif kb == 8:
                        z_alloc(2)
                    if kb >= 8:
                        z_group(2, 2 * (kb - 8))
                        z_group(2, 2 * (kb - 8) + 1)
                else:
                    if kb == 0:
                        norm(2)
                    elif kb == 1:
                        z_alloc(3)
                    if kb >= 1:
                        z_group(3, kb - 1)
            if pair == NFT - 1:
                z_group(3, 15)
                norm(3)

        if dbg:
            nc.sync.dma_start(dbg_out["KT0"][:], k_t[0][:])
            nc.sync.dma_start(dbg_out["QT0"][:], q_t[0][:])
            nc.sync.dma_start(dbg_out["VA0"][:], v_aug[0][:])
            nc.sync.dma_start(dbg_out["XKT0"][:], xkt[0][:])
            nc.sync.dma_start(dbg_out["ZT0"][:], z_t[0][:])

        # ---- output projection ----
        for qb in range(NQB):
            po = ps_tile([128, D], "ab", f"po{qb}")
            for p4 in range(NFT):
                nc.tensor.matmul(
                    po[:],
                    z_t[p4][:, 128 * qb:128 * (qb + 1)],
                    wo_t[p4][:],
                    start=(p4 == 0),
                    stop=(p4 == NFT - 1),
                )
            ot = out_pool.tile([128, D], F32, tag="ot")
            nc.any.tensor_add(ot[:], po[:], bo_bc[:])
            nc.sync.dma_start(out[128 * qb:128 * (qb + 1), :], ot[:])

    nc.compile()
    return nc


_NC = None
LAST_RESULTS = None


def _get_nc():
    global _NC
    if _NC is None:
        _NC = build_program(dbg=bool(int(os.environ.get("KERNEL_DEBUG", "0"))))
    return _NC


def _bf(x):
    return np.ascontiguousarray(np.asarray(x).astype(ml_dtypes.bfloat16))


def kernel(Q, K, V, Wq, bq, Wk, bk, Wv, bv, Wo, bo):
    global LAST_RESULTS
    nc = _get_nc()
    Qb, Kb, Vb = _bf(Q), _bf(K), _bf(V)
    shared = {
        "Wq": _bf(Wq),
        "Wk": _bf(Wk),
        "Wv": _bf(Wv),
        "Wo": _bf(Wo),
        "bq": np.ascontiguousarray(np.asarray(bq, np.float32).reshape(D, 1)),
        "bk": np.ascontiguousarray(np.asarray(bk, np.float32).reshape(D, 1)),
        "bv": np.ascontiguousarray(np.asarray(bv, np.float32).reshape(1, D)),
        "bo": np.ascontiguousarray(np.asarray(bo, np.float32).reshape(1, D)),
    }
    in_maps = []
    for c in range(N_CORES):
        b, qh = c // 2, c % 2
        in_maps.append({
            "XQ": np.ascontiguousarray(Qb[b, SQ * qh:SQ * (qh + 1)]),
            "XK": Kb[b],
            "XV": Vb[b],
            **shared,
        })
    trace = bool(int(os.environ.get("KERNEL_TRACE", "0")))
    res = bass_utils.run_bass_kernel_spmd(
        nc, in_maps, core_ids=list(range(N_CORES)), trace=trace,
    )
    LAST_RESULTS = res
    out = np.empty((B, S, D), dtype=np.float32)
    for c in range(N_CORES):
        b, qh = c // 2, c % 2
        out[b, SQ * qh:SQ * (qh + 1)] = res.results[c]["OUT"]
    return out



# revision 12
# speedup vs baseline: 1.0986x; 1.0986x over previous
"""Trainium2 Bass kernel for nn_MultiHeadAttention (B=4, S=2048, D=512, H=8, DH=64).

Sharding: 8 cores = 4 batches x 2 head-groups (tensor parallel over heads).
Each core projects Q/K/V for its 4 heads over the full 2048 rows (no
duplicated projection work), runs attention for those heads, and computes a
partial output projection (contraction over its 256 z-features). The host
sums the two partial outputs per batch.

Per-core pipeline (bf16 datapath, fp32 PSUM accumulation), organized as 128
uniform slots = 8 groups (4 q-blocks x 2 head-pairs) x 16 key-blocks:
  slot: S^T[k,1024] = [KA^T|KB^T](stationary) @ Q^T  (two 512-col matmuls,
        heads A/B side by side in one 2-bank PSUM tile, double-buffered)
        -> one ScalarE exp per slot ([128,1024] PSUM -> SBUF bf16)
        -> z^T[65,512] += [V_h|1](stationary) @ P^T   (lagged 4 slots)
  ScalarE does nothing but exp (the binding engine, ~131us); projections ride
  the PE slack as 512-col chunks through 2 spare PSUM banks; all bias adds,
  norm copies/mults on DVE; partition broadcasts on GpSimd.
"""

import os
import sys

import numpy as np

sys.path.insert(0, "/opt/trn_rl_repo")

import ml_dtypes
import concourse.bacc as bacc
import concourse.bass as bass
import concourse.mybir as mybir
import concourse.tile as tile
from concourse import bass_utils

F32 = mybir.dt.float32
BF16 = mybir.dt.bfloat16

B, S, D, H, DH = 4, 2048, 512, 8, 64
HG = H // 2          # heads per core (head-group)
DG = HG * DH         # 256 features per core
NKB = S // 128       # 16 k-blocks
NMC = D // 128       # 4 input-feature chunks
NQH = S // 512       # 4 query blocks of 512
NPAIR = HG // 2      # 2 head pairs per core
N_CORES = 8

Exp = mybir.ActivationFunctionType.Exp


def build_program(dbg=False):
    nc = bacc.Bacc("TRN2", target_bir_lowering=False, debug=False)
    dbg_out = {}
    if dbg:
        for nm, shp, dt in [("KT0", [128, S], BF16), ("QT0", [128, S], BF16),
                            ("VA0", [128, 260], BF16), ("P000", [128, 1024], BF16),
                            ("ZT0", [128, S], BF16), ("ZT1", [128, S], BF16),
                            ("ZC000", [65, 512], F32), ("RC000", [1, 512], F32),
                            ("ZC001", [65, 512], F32), ("RC001", [1, 512], F32)]:
            dbg_out[nm] = nc.dram_tensor(nm, shp, dt, kind="ExternalOutput").ap()

    xq = nc.dram_tensor("XQ", [S, D], BF16, kind="ExternalInput").ap()
    xk = nc.dram_tensor("XK", [S, D], BF16, kind="ExternalInput").ap()
    xv = nc.dram_tensor("XV", [S, D], BF16, kind="ExternalInput").ap()
    wq = nc.dram_tensor("Wq", [D, DG], BF16, kind="ExternalInput").ap()
    wk = nc.dram_tensor("Wk", [D, DG], BF16, kind="ExternalInput").ap()
    wv = nc.dram_tensor("Wv", [D, DG], BF16, kind="ExternalInput").ap()
    wo = nc.dram_tensor("Wo", [DG, D], BF16, kind="ExternalInput").ap()
    bq = nc.dram_tensor("bq", [DG, 1], F32, kind="ExternalInput").ap()
    bk = nc.dram_tensor("bk", [DG, 1], F32, kind="ExternalInput").ap()
    bv = nc.dram_tensor("bv", [1, DG], F32, kind="ExternalInput").ap()
    bo = nc.dram_tensor("bo", [1, D], F32, kind="ExternalInput").ap()
    out = nc.dram_tensor("OUT", [S, D], F32, kind="ExternalOutput").ap()

    from contextlib import ExitStack

    with tile.TileContext(nc) as tc, ExitStack() as ctx:
        const = ctx.enter_context(tc.tile_pool(name="const", bufs=1))
        xt_pool = ctx.enter_context(tc.tile_pool(name="xt", bufs=1))
        w_pool = ctx.enter_context(tc.tile_pool(name="w", bufs=1))
        kt_pool = ctx.enter_context(tc.tile_pool(name="kt", bufs=1))
        qt_pool = ctx.enter_context(tc.tile_pool(name="qt", bufs=1))
        v_pool = ctx.enter_context(tc.tile_pool(name="v", bufs=1))
        p_pool = ctx.enter_context(tc.tile_pool(name="p", bufs=6))
        zt_pool = ctx.enter_context(tc.tile_pool(name="zt", bufs=1))
        nrm_pool = ctx.enter_context(tc.tile_pool(name="nrm", bufs=3))
        out_pool = ctx.enter_context(tc.tile_pool(name="outp", bufs=2))

        # PSUM: s0/s1 [128,1024] (2 banks each), zA/zB [65->128,512] (1 bank
        # each), pr0/pr1 [128,512] (1 bank each) = 8 banks exactly.
        ps = ctx.enter_context(tc.tile_pool(name="ps", bufs=1, space="PSUM"))
        pr_ctr = [0]

        def pr_tile(cols, name):
            tag = f"pr{pr_ctr[0] % 2}"
            pr_ctr[0] += 1
            return ps.tile([128, cols], F32, tag=tag, name=name,
                           padded_shape=[128, 512])

        # ---- warm the ScalarE Exp table immediately ----
        warm = nrm_pool.tile([1, 8], F32, tag="warm")
        nc.gpsimd.memset(warm[:], 0.0)
        warm2 = nrm_pool.tile([1, 8], F32, tag="warm2")
        nc.scalar.activation(warm2[:], warm[:], Exp, scale=0.125)

        # ---- weight loads: one DMA per tensor ----
        def load_w(wdram, name, cols, nch=NMC):
            big = w_pool.tile([128, nch * cols], BF16, tag=f"w{name}",
                              name=f"w{name}")
            nc.sync.dma_start(
                big[:].rearrange("p (g c) -> p g c", g=nch),
                wdram.rearrange("(g p) c -> p g c", p=128),
            )
            return [big[:, cols * mc:cols * (mc + 1)] for mc in range(nch)]

        # ---- X^T via DMA xbar transpose; first 512 cols split out so the
        # first projection chunks can start early ----
        def load_xt(xdram, name, split_first):
            tiles = []
            for mc in range(NMC):
                t = xt_pool.tile([128, S], BF16, tag=f"xt{name}{mc}",
                                 name=f"xt{name}{mc}")
                src = xdram[:, 128 * mc:128 * (mc + 1)]
                if split_first:
                    nc.sync.dma_start(t[:, 0:512], src[0:512, :],
                                      transpose=True)
                    nc.sync.dma_start(t[:, 512:S], src[512:S, :],
                                      transpose=True)
                else:
                    nc.sync.dma_start(t[:], src, transpose=True)
                tiles.append(t)
            return tiles

        wk_t = load_w(wk, "k", DG)
        xkt = load_xt(xk, "k", split_first=True)
        wq_t = load_w(wq, "q", DG)
        xqt = load_xt(xq, "q", split_first=True)

        bq_all = const.tile([128, NPAIR], F32, tag="bqa")
        nc.sync.dma_start(
            bq_all[:].rearrange("p (g o) -> p g o", g=NPAIR),
            bq.rearrange("(g p) o -> p g o", p=128),
        )
        bk_all = const.tile([128, NPAIR], F32, tag="bka")
        nc.sync.dma_start(
            bk_all[:].rearrange("p (g o) -> p g o", g=NPAIR),
            bk.rearrange("(g p) o -> p g o", p=128),
        )

        wv_t = load_w(wv, "v", DG)
        xvt = load_xt(xv, "v", split_first=False)
        wo_t = load_w(wo, "o", D, nch=NPAIR)

        bv_row = const.tile([1, DG], F32, tag="bvr")
        nc.sync.dma_start(bv_row[:], bv[:])
        bv_bc = const.tile([128, DG], F32, tag="bvb")
        nc.gpsimd.partition_broadcast(bv_bc[:], bv_row[:], channels=128)
        bo_row = const.tile([1, D], F32, tag="bor")
        nc.sync.dma_start(bo_row[:], bo[:])
        bo_bc = const.tile([128, D], F32, tag="bob")
        nc.gpsimd.partition_broadcast(bo_bc[:], bo_row[:], channels=128)

        # ---- persistent SBUF results ----
        k_t = [kt_pool.tile([128, S], BF16, tag=f"kt{p}", name=f"kt{p}")
               for p in range(NPAIR)]
        q_t = [qt_pool.tile([128, S], BF16, tag=f"qt{p}", name=f"qt{p}")
               for p in range(NPAIR)]
        VW = HG * (DH + 1)  # 260: per head 64 value cols + 1 ones col
        v_aug = [v_pool.tile([128, VW], BF16, tag=f"v{kb}", name=f"v{kb}")
                 for kb in range(NKB)]
        z_t = [zt_pool.tile([128, S], BF16, tag=f"zt{p}", name=f"zt{p}")
               for p in range(NPAIR)]

        # ---- projection chunks (feeder work) ----
        def kq_chunk(dst, w_ts, b_all, pair, cb):
            pj = pr_tile(512, f"pj{cb}")
            for mc in range(NMC):
                nc.tensor.matmul(
                    pj[:],
                    w_ts[mc][:, 128 * pair:128 * (pair + 1)],
                    (xkt if dst is k_t else xqt)[mc][:, 512 * cb:512 * (cb + 1)],
                    start=(mc == 0),
                    stop=(mc == NMC - 1),
                )
            nc.vector.tensor_scalar_add(
                dst[pair][:, 512 * cb:512 * (cb + 1)], pj[:],
                b_all[:, pair:pair + 1],
            )

        def v_chunk(kb):
            nc.gpsimd.memset(
                v_aug[kb][:].rearrange("p (h c) -> p h c", h=HG)[:, :, DH:DH + 1],
                1.0,
            )
            pj = pr_tile(DG, f"pjv{kb}")
            for mc in range(NMC):
                nc.tensor.matmul(
                    pj[:],
                    xvt[mc][:, 128 * kb:128 * (kb + 1)],
                    wv_t[mc][:],
                    start=(mc == 0),
                    stop=(mc == NMC - 1),
                )
            nc.vector.tensor_add(
                v_aug[kb][:].rearrange("p (h c) -> p h c", h=HG)[:, :, 0:DH],
                pj[:].rearrange("p (h c) -> p h c", h=HG),
                bv_bc[:].rearrange("p (h c) -> p h c", h=HG),
            )

        def o_chunk(qh, qc):
            po = pr_tile(512, f"po{qh}{qc}")
            qs = slice(512 * qh + 128 * qc, 512 * qh + 128 * (qc + 1))
            for p2 in range(NPAIR):
                nc.tensor.matmul(
                    po[:],
                    z_t[p2][:, qs],
                    wo_t[p2][:],
                    start=(p2 == 0),
                    stop=(p2 == NPAIR - 1),
                )
            ot = out_pool.tile([128, D], F32, tag="ot")
            nc.vector.tensor_add(ot[:], po[:], bo_bc[:])
            nc.sync.dma_start(out[qs, :], ot[:])

        # ---- attention slot machinery ----
        p_slabs = {}
        z_tiles = [None]

        def s_slot(qh, pair, kb, sl):
            st = ps.tile([128, 1024], F32, tag=f"s{sl % 2}",
                         name=f"s{qh}_{pair}_{kb}", padded_shape=[128, 1024])
            qs = slice(512 * qh, 512 * (qh + 1))
            ks = slice(128 * kb, 128 * (kb + 1))
            nc.tensor.matmul(st[:, 0:512], k_t[pair][0:DH, ks],
                             q_t[pair][0:DH, qs],
                             start=True, stop=True, tile_position=(0, 0))
            nc.tensor.matmul(st[:, 512:1024], k_t[pair][DH:128, ks],
                             q_t[pair][DH:128, qs],
                             start=True, stop=True, tile_position=(64, 0))
            pab = p_pool.tile([128, 1024], BF16, tag="p",
                              name=f"p{qh}_{pair}_{kb}")
            nc.scalar.activation(pab[:], st[:], Exp, scale=0.125)
            if dbg and (qh, pair, kb) == (0, 0, 0):
                nc.sync.dma_start(dbg_out["P000"][:], pab[:])
            p_slabs[(qh, pair, kb)] = pab

        def z_alloc():
            zA = ps.tile([DH + 1, 512], F32, tag="zA", name="zA",
                         padded_shape=[128, 512])
            zB = ps.tile([DH + 1, 512], F32, tag="zB", name="zB",
                         padded_shape=[128, 512])
            z_tiles[0] = (zA, zB)

        def z_group(qh, pair, kb):
            if kb == 0:
                z_alloc()
            zA, zB = z_tiles[0]
            pab = p_slabs.pop((qh, pair, kb))
            hA, hB = 2 * pair, 2 * pair + 1
            nc.tensor.matmul(zA[:], v_aug[kb][:, 65 * hA:65 * hA + 65],
                             pab[:, 0:512],
                             start=(kb == 0), stop=(kb == NKB - 1),
                             skip_group_check=True)
            nc.tensor.matmul(zB[:], v_aug[kb][:, 65 * hB:65 * hB + 65],
                             pab[:, 512:1024],
                             start=(kb == 0), stop=(kb == NKB - 1),
                             skip_group_check=True)

        def norm(qh, pair):
            zA, zB = z_tiles[0]
            for z_ps, half in ((zA, 0), (zB, 1)):
                # copy PSUM->SBUF first so the z banks free up fast
                zc = nrm_pool.tile([DH + 1, 512], F32, tag="zc")
                nc.vector.tensor_copy(zc[:], z_ps[:])
                # custom-DVE recip needs a fresh partition-0 source tile
                row = nrm_pool.tile([1, 512], F32, tag="row")
                nc.vector.tensor_copy(row[:], zc[DH:DH + 1, :])
                recip = nrm_pool.tile([1, 512], F32, tag="recip")
                nc.vector.reciprocal_approx_fast(recip[:], row[:])
                rbc = nrm_pool.tile([DH, 512], F32, tag="rbc")
                nc.gpsimd.partition_broadcast(rbc[:], recip[:], channels=DH)
                nc.vector.tensor_mul(
                    z_t[pair][64 * half:64 * half + 64,
                              512 * qh:512 * (qh + 1)],
                    zc[0:DH, :], rbc[:],
                )
                if dbg and (qh, pair) == (0, 0):
                    nc.sync.dma_start(dbg_out[f"ZC00{half}"][:], zc[:])
                    nc.sync.dma_start(dbg_out[f"RC00{half}"][:], recip[:])

        # ---- feeder schedule (ordered by first-use time with margin) ----
        def kq(dst, w_ts, b_all, pair, cb):
            return lambda: kq_chunk(dst, w_ts, b_all, pair, cb)

        def vch(kb):
            return lambda: v_chunk(kb)

        feeder = [
            kq(q_t, wq_t, bq_all, 1, 0),   # group 1 slot 0 needs this
            vch(2),
            kq(k_t, wk_t, bk_all, 0, 1),
            vch(3),
            kq(k_t, wk_t, bk_all, 0, 2),
            vch(4),
            kq(k_t, wk_t, bk_all, 0, 3),
            vch(5), vch(6), vch(7),
            kq(k_t, wk_t, bk_all, 1, 0),
            vch(8),
            kq(k_t, wk_t, bk_all, 1, 1),
            vch(9),
            kq(k_t, wk_t, bk_all, 1, 2),
            vch(10),
            kq(k_t, wk_t, bk_all, 1, 3),
            vch(11), vch(12), vch(13), vch(14), vch(15),
        ]
        for qh in range(1, NQH):
            for pair in range(NPAIR):
                feeder.append(kq(q_t, wq_t, bq_all, pair, qh))
        fi = [0]
        o_queue = []
        oi = [0]

        def feed(n):
            for _ in range(n):
                if fi[0] < len(feeder):
                    feeder[fi[0]]()
                    fi[0] += 1

        def feed_o(n):
            for _ in range(n):
                if oi[0] < len(o_queue):
                    o_queue[oi[0]]()
                    oi[0] += 1

        # ---- ramp: minimum to start slot 0 ----
        kq_chunk(k_t, wk_t, bk_all, 0, 0)
        kq_chunk(q_t, wq_t, bq_all, 0, 0)
        v_chunk(0)
        v_chunk(1)

        # ---- main loop: 8 groups x 16 slots ----
        groups = [(qh, pair) for qh in range(NQH) for pair in range(NPAIR)]
        Z_LAG = 4
        for gi, (qh, pair) in enumerate(groups):
            for t in range(NKB):
                s_slot(qh, pair, t, gi * NKB + t)
                if t >= Z_LAG:
                    z_group(qh, pair, t - Z_LAG)
                elif gi > 0 and t < Z_LAG:
                    pqh, ppair = groups[gi - 1]
                    z_group(pqh, ppair, NKB - Z_LAG + t)
                    if t == Z_LAG - 1:
                        norm(pqh, ppair)
                        if ppair == NPAIR - 1:
                            # schedule output chunks for the finished q-block
                            for qc in range(4):
                                o_queue.append(
                                    (lambda a, b: (lambda: o_chunk(a, b)))
                                    (pqh, qc))
                # feed: slots 0-3 have at most one z pair -> extra capacity
                feed(2 if t < Z_LAG else 1)
                # output chunks only after the norm chain has fully drained
                if t in (8, 10, 12, 14):
                    feed_o(1)

        # ---- drain: last group's tail z, norm, and final output chunks ----
        lqh, lpair = groups[-1]
        for t in range(Z_LAG):
            z_group(lqh, lpair, NKB - Z_LAG + t)
        norm(lqh, lpair)
        feed(len(feeder))
        feed_o(len(o_queue))
        for qc in range(4):
            o_chunk(lqh, qc)

        if dbg:
            nc.sync.dma_start(dbg_out["KT0"][:], k_t[0][:])
            nc.sync.dma_start(dbg_out["QT0"][:], q_t[0][:])
            nc.sync.dma_start(dbg_out["VA0"][:], v_aug[0][:])
            nc.sync.dma_start(dbg_out["ZT0"][:], z_t[0][:])
            nc.sync.dma_start(dbg_out["ZT1"][:], z_t[1][:])

    nc.compile()
    return nc


_NC = None
LAST_RESULTS = None


def _get_nc():
    global _NC
    if _NC is None:
        _NC = build_program(dbg=bool(int(os.environ.get("KERNEL_DEBUG", "0"))))
    return _NC


def _bf(x):
    return np.ascontiguousarray(np.asarray(x).astype(ml_dtypes.bfloat16))


def kernel(Q, K, V, Wq, bq, Wk, bk, Wv, bv, Wo, bo):
    global LAST_RESULTS
    nc = _get_nc()
    Qb, Kb, Vb = _bf(Q), _bf(K), _bf(V)
    Wqb, Wkb, Wvb, Wob = _bf(Wq), _bf(Wk), _bf(Wv), _bf(Wo)
    bqf = np.asarray(bq, np.float32)
    bkf = np.asarray(bk, np.float32)
    bvf = np.asarray(bv, np.float32)
    bof = np.asarray(bo, np.float32)
    in_maps = []
    for c in range(N_CORES):
        b, hg = c // 2, c % 2
        fs = slice(DG * hg, DG * (hg + 1))
        in_maps.append({
            "XQ": Qb[b],
            "XK": Kb[b],
            "XV": Vb[b],
            "Wq": np.ascontiguousarray(Wqb[:, fs]),
            "Wk": np.ascontiguousarray(Wkb[:, fs]),
            "Wv": np.ascontiguousarray(Wvb[:, fs]),
            "Wo": np.ascontiguousarray(Wob[fs, :]),
            "bq": np.ascontiguousarray(bqf[fs].reshape(DG, 1)),
            "bk": np.ascontiguousarray(bkf[fs].reshape(DG, 1)),
            "bv": np.ascontiguousarray(bvf[fs].reshape(1, DG)),
            "bo": np.ascontiguousarray((bof * 0.5).reshape(1, D)),
        })
    trace = bool(int(os.environ.get("KERNEL_TRACE", "0")))
    res = bass_utils.run_bass_kernel_spmd(
        nc, in_maps, core_ids=list(range(N_CORES)), trace=trace,
    )
    LAST_RESULTS = res
    out = np.empty((B, S, D), dtype=np.float32)
    for b in range(B):
        out[b] = res.results[2 * b]["OUT"] + res.results[2 * b + 1]["OUT"]
    return out


# revision 23
# speedup vs baseline: 1.1471x; 1.0441x over previous
"""Trainium2 Bass kernel for nn_MultiHeadAttention (B=4, S=2048, D=512, H=8, DH=64).

Sharding: 8 cores = 4 batches x 2 head-groups (tensor parallel over heads).
Each core projects Q/K/V for its 4 heads over the full 2048 rows (no
duplicated projection work), runs attention for those heads, and computes a
partial output projection (contraction over its 256 z-features). The host
sums the two partial outputs per batch.

Per-core pipeline (bf16 datapath, fp32 PSUM accumulation), organized as 128
uniform slots = 8 groups (4 q-blocks x 2 head-pairs) x 16 key-blocks:
  slot: S^T[k,1024] = [KA^T|KB^T](stationary) @ Q^T  (two 512-col matmuls,
        heads A/B side by side in one 2-bank PSUM tile, double-buffered)
        -> one ScalarE exp per slot ([128,1024] PSUM -> SBUF bf16)
        -> z^T[65,512] += [V_h|1](stationary) @ P^T   (lagged 4 slots)
  ScalarE does nothing but exp (the binding engine, ~131us); projections ride
  the PE slack as 512-col chunks through 2 spare PSUM banks; all bias adds,
  norm copies/mults on DVE; partition broadcasts on GpSimd.
"""

import os
import sys

import numpy as np

sys.path.insert(0, "/opt/trn_rl_repo")

import ml_dtypes
import concourse.bacc as bacc
import concourse.bass as bass
import concourse.mybir as mybir
import concourse.tile as tile
from concourse import bass_utils

F32 = mybir.dt.float32
BF16 = mybir.dt.bfloat16

B, S, D, H, DH = 4, 2048, 512, 8, 64
HG = H // 2          # heads per core (head-group)
DG = HG * DH         # 256 features per core
NKB = S // 128       # 16 k-blocks
NMC = D // 128       # 4 input-feature chunks
NQH = S // 512       # 4 query blocks of 512
NPAIR = HG // 2      # 2 head pairs per core
N_CORES = 8

Exp = mybir.ActivationFunctionType.Exp


def build_program(dbg=False):
    nc = bacc.Bacc("TRN2", target_bir_lowering=False, debug=False)
    dbg_out = {}
    if dbg:
        for nm, shp, dt in [("KT0", [128, S], BF16), ("QT0", [128, S], BF16),
                            ("VA0", [128, 260], BF16), ("P000", [128, 1024], BF16),
                            ("ZT0", [128, S], BF16), ("ZT1", [128, S], BF16),
                            ("ZC000", [65, 512], F32), ("RC000", [1, 512], F32),
                            ("ZC001", [65, 512], F32), ("RC001", [1, 512], F32)]:
            dbg_out[nm] = nc.dram_tensor(nm, shp, dt, kind="ExternalOutput").ap()

    xq = nc.dram_tensor("XQ", [S, D], BF16, kind="ExternalInput").ap()
    xk = nc.dram_tensor("XK", [S, D], BF16, kind="ExternalInput").ap()
    xv = nc.dram_tensor("XV", [S, D], BF16, kind="ExternalInput").ap()
    wq = nc.dram_tensor("Wq", [D, DG], BF16, kind="ExternalInput").ap()
    wk = nc.dram_tensor("Wk", [D, DG], BF16, kind="ExternalInput").ap()
    wv = nc.dram_tensor("Wv", [D, DG], BF16, kind="ExternalInput").ap()
    wo = nc.dram_tensor("Wo", [DG, D], BF16, kind="ExternalInput").ap()
    bq = nc.dram_tensor("bq", [DG, 1], F32, kind="ExternalInput").ap()
    bk = nc.dram_tensor("bk", [DG, 1], F32, kind="ExternalInput").ap()
    bv = nc.dram_tensor("bv", [1, DG], F32, kind="ExternalInput").ap()
    bo = nc.dram_tensor("bo", [1, D], F32, kind="ExternalInput").ap()
    out = nc.dram_tensor("OUT", [S, D], F32, kind="ExternalOutput").ap()

    from contextlib import ExitStack

    with tile.TileContext(nc) as tc, ExitStack() as ctx:
        const = ctx.enter_context(tc.tile_pool(name="const", bufs=1))
        xt_pool = ctx.enter_context(tc.tile_pool(name="xt", bufs=1))
        w_pool = ctx.enter_context(tc.tile_pool(name="w", bufs=1))
        kt_pool = ctx.enter_context(tc.tile_pool(name="kt", bufs=1))
        qt_pool = ctx.enter_context(tc.tile_pool(name="qt", bufs=1))
        v_pool = ctx.enter_context(tc.tile_pool(name="v", bufs=1))
        p_pool = ctx.enter_context(tc.tile_pool(name="p", bufs=6))
        zt_pool = ctx.enter_context(tc.tile_pool(name="zt", bufs=1))
        nrm_pool = ctx.enter_context(tc.tile_pool(name="nrm", bufs=3))
        out_pool = ctx.enter_context(tc.tile_pool(name="outp", bufs=2))

        # PSUM: s0/s1 [128,1024] (2 banks each), zA/zB [65->128,512] (1 bank
        # each), pr0/pr1 [128,512] (1 bank each) = 8 banks exactly.
        ps = ctx.enter_context(tc.tile_pool(name="ps", bufs=1, space="PSUM"))
        pr_ctr = [0]

        def pr_tile(cols, name):
            tag = f"pr{pr_ctr[0] % 2}"
            pr_ctr[0] += 1
            return ps.tile([128, cols], F32, tag=tag, name=name,
                           padded_shape=[128, 512])

        # ---- warm the ScalarE Exp table immediately ----
        warm = nrm_pool.tile([1, 8], F32, tag="warm")
        nc.gpsimd.memset(warm[:], 0.0)
        warm2 = nrm_pool.tile([1, 8], F32, tag="warm2")
        nc.scalar.activation(warm2[:], warm[:], Exp, scale=0.125)

        # ---- weight loads: one DMA per tensor (sync queue) ----
        def load_w(wdram, name, cols, nch=NMC, eng=None):
            big = w_pool.tile([128, nch * cols], BF16, tag=f"w{name}",
                              name=f"w{name}")
            (eng or nc.sync).dma_start(
                big[:].rearrange("p (g c) -> p g c", g=nch),
                wdram.rearrange("(g p) c -> p g c", p=128),
            )
            return [big[:, cols * mc:cols * (mc + 1)] for mc in range(nch)]

        # ---- X^T via DMA xbar transpose, issue spread across engine queues
        # (each issue costs ~1.5us on the issuing engine; serializing all of
        # them on sync delays the pipeline start by ~30us) ----
        def load_xt(xdram, name, eng):
            tiles = []
            for mc in range(NMC):
                t = xt_pool.tile([128, S], BF16, tag=f"xt{name}{mc}",
                                 name=f"xt{name}{mc}")
                eng.dma_start(t[:], xdram[:, 128 * mc:128 * (mc + 1)],
                              transpose=True)
                tiles.append(t)
            return tiles

        # tiny bias DMAs first so the first bias-adds are never blocked
        bq_all = const.tile([128, NPAIR], F32, tag="bqa")
        nc.sync.dma_start(
            bq_all[:].rearrange("p (g o) -> p g o", g=NPAIR),
            bq.rearrange("(g p) o -> p g o", p=128),
        )
        bk_all = const.tile([128, NPAIR], F32, tag="bka")
        nc.sync.dma_start(
            bk_all[:].rearrange("p (g o) -> p g o", g=NPAIR),
            bk.rearrange("(g p) o -> p g o", p=128),
        )
        bv_row = const.tile([1, DG], F32, tag="bvr")
        nc.sync.dma_start(bv_row[:], bv[:])
        bo_row = const.tile([1, D], F32, tag="bor")
        nc.sync.dma_start(bo_row[:], bo[:])

        # all input DMAs on the sync HWDGE queue, K before Q before V so the
        # pipeline's first consumers unblock earliest
        wk_t = load_w(wk, "k", DG)
        xkt = load_xt(xk, "k", nc.sync)
        wq_t = load_w(wq, "q", DG)
        xqt = load_xt(xq, "q", nc.sync)
        wv_t = load_w(wv, "v", DG)
        xvt = load_xt(xv, "v", nc.sync)
        wo_t = load_w(wo, "o", D, nch=NPAIR)

        bv_bc = const.tile([128, DG], F32, tag="bvb")
        nc.gpsimd.partition_broadcast(bv_bc[:], bv_row[:], channels=128)
        bo_bc = const.tile([128, D], F32, tag="bob")
        nc.gpsimd.partition_broadcast(bo_bc[:], bo_row[:], channels=128)

        # ---- persistent SBUF results ----
        k_t = [kt_pool.tile([128, S], BF16, tag=f"kt{p}", name=f"kt{p}")
               for p in range(NPAIR)]
        q_t = [qt_pool.tile([128, S], BF16, tag=f"qt{p}", name=f"qt{p}")
               for p in range(NPAIR)]
        VW = HG * (DH + 1)  # 260: per head 64 value cols + 1 ones col
        v_aug = [v_pool.tile([128, VW], BF16, tag=f"v{kb}", name=f"v{kb}")
                 for kb in range(NKB)]
        z_t = [zt_pool.tile([128, S], BF16, tag=f"zt{p}", name=f"zt{p}")
               for p in range(NPAIR)]

        # ---- projection chunks (feeder work) ----
        def kq_chunk(dst, w_ts, b_all, pair, cb):
            pj = pr_tile(512, f"pj{cb}")
            for mc in range(NMC):
                nc.tensor.matmul(
                    pj[:],
                    w_ts[mc][:, 128 * pair:128 * (pair + 1)],
                    (xkt if dst is k_t else xqt)[mc][:, 512 * cb:512 * (cb + 1)],
                    start=(mc == 0),
                    stop=(mc == NMC - 1),
                )
            nc.vector.tensor_scalar_add(
                dst[pair][:, 512 * cb:512 * (cb + 1)], pj[:],
                b_all[:, pair:pair + 1],
            )

        def v_chunk(kb):
            nc.gpsimd.memset(
                v_aug[kb][:].rearrange("p (h c) -> p h c", h=HG)[:, :, DH:DH + 1],
                1.0,
            )
            pj = pr_tile(DG, f"pjv{kb}")
            for mc in range(NMC):
                nc.tensor.matmul(
                    pj[:],
                    xvt[mc][:, 128 * kb:128 * (kb + 1)],
                    wv_t[mc][:],
                    start=(mc == 0),
                    stop=(mc == NMC - 1),
                )
            nc.vector.tensor_add(
                v_aug[kb][:].rearrange("p (h c) -> p h c", h=HG)[:, :, 0:DH],
                pj[:].rearrange("p (h c) -> p h c", h=HG),
                bv_bc[:].rearrange("p (h c) -> p h c", h=HG),
            )

        def o_chunk(qh, qc):
            po = pr_tile(512, f"po{qh}{qc}")
            qs = slice(512 * qh + 128 * qc, 512 * qh + 128 * (qc + 1))
            for p2 in range(NPAIR):
                nc.tensor.matmul(
                    po[:],
                    z_t[p2][:, qs],
                    wo_t[p2][:],
                    start=(p2 == 0),
                    stop=(p2 == NPAIR - 1),
                )
            ot = out_pool.tile([128, D], F32, tag="ot")
            nc.vector.tensor_add(ot[:], po[:], bo_bc[:])
            nc.sync.dma_start(out[qs, :], ot[:])

        # ---- attention slot machinery ----
        p_slabs = {}
        z_tiles = [None]

        def s_slot(qh, pair, kb, sl):
            st = ps.tile([128, 1024], F32, tag=f"s{sl % 2}",
                         name=f"s{qh}_{pair}_{kb}", padded_shape=[128, 1024])
            qs = slice(512 * qh, 512 * (qh + 1))
            ks = slice(128 * kb, 128 * (kb + 1))
            nc.tensor.matmul(st[:, 0:512], k_t[pair][0:DH, ks],
                             q_t[pair][0:DH, qs],
                             start=True, stop=True, tile_position=(0, 0))
            nc.tensor.matmul(st[:, 512:1024], k_t[pair][DH:128, ks],
                             q_t[pair][DH:128, qs],
                             start=True, stop=True, tile_position=(64, 0))
            pab = p_pool.tile([128, 1024], BF16, tag="p",
                              name=f"p{qh}_{pair}_{kb}")
            nc.scalar.activation(pab[:], st[:], Exp, scale=0.125)
            if dbg and (qh, pair, kb) == (0, 0, 0):
                nc.sync.dma_start(dbg_out["P000"][:], pab[:])
            p_slabs[(qh, pair, kb)] = pab

        def z_alloc():
            zA = ps.tile([DH + 1, 512], F32, tag="zA", name="zA",
                         padded_shape=[128, 512])
            zB = ps.tile([DH + 1, 512], F32, tag="zB", name="zB",
                         padded_shape=[128, 512])
            z_tiles[0] = (zA, zB)

        def z_group(qh, pair, kb):
            if kb == 0:
                z_alloc()
            zA, zB = z_tiles[0]
            pab = p_slabs.pop((qh, pair, kb))
            hA, hB = 2 * pair, 2 * pair + 1
            nc.tensor.matmul(zA[:], v_aug[kb][:, 65 * hA:65 * hA + 65],
                             pab[:, 0:512],
                             start=(kb == 0), stop=(kb == NKB - 1),
                             skip_group_check=True)
            nc.tensor.matmul(zB[:], v_aug[kb][:, 65 * hB:65 * hB + 65],
                             pab[:, 512:1024],
                             start=(kb == 0), stop=(kb == NKB - 1),
                             skip_group_check=True)

        def norm(qh, pair):
            zA, zB = z_tiles[0]
            zcs = []
            for z_ps, half in ((zA, 0), (zB, 1)):
                # copy PSUM->SBUF first so both z banks free up fast
                zc = nrm_pool.tile([DH + 1, 512], F32, tag=f"zc{half}")
                nc.vector.tensor_copy(zc[:], z_ps[:])
                zcs.append(zc)
            for zc, half in zip(zcs, (0, 1)):
                # custom-DVE recip needs a fresh partition-0 source tile
                row = nrm_pool.tile([1, 512], F32, tag="row")
                nc.vector.tensor_copy(row[:], zc[DH:DH + 1, :])
                recip = nrm_pool.tile([1, 512], F32, tag="recip")
                nc.vector.reciprocal_approx_fast(recip[:], row[:])
                rbc = nrm_pool.tile([DH, 512], F32, tag="rbc")
                nc.gpsimd.partition_broadcast(rbc[:], recip[:], channels=DH)
                nc.vector.tensor_mul(
                    z_t[pair][64 * half:64 * half + 64,
                              512 * qh:512 * (qh + 1)],
                    zc[0:DH, :], rbc[:],
                )
                if dbg and (qh, pair) == (0, 0):
                    nc.sync.dma_start(dbg_out[f"ZC00{half}"][:], zc[:])
                    nc.sync.dma_start(dbg_out[f"RC00{half}"][:], recip[:])

        # ---- feeder schedule (ordered by first-use time with margin) ----
        def kq(dst, w_ts, b_all, pair, cb):
            return lambda: kq_chunk(dst, w_ts, b_all, pair, cb)

        def vch(kb):
            return lambda: v_chunk(kb)

        # ordered by first-use slot; group 0 consumes 2/slot in slots 0-3,
        # then 1/slot (see feed calls below)
        feeder = [
            kq(k_t, wk_t, bk_all, 0, 1), vch(0),
            kq(q_t, wq_t, bq_all, 1, 0), vch(1),
            kq(k_t, wk_t, bk_all, 0, 2), vch(2),
            kq(k_t, wk_t, bk_all, 0, 3), vch(3),
            vch(4), vch(5), vch(6), vch(7), vch(8), vch(9), vch(10),
            kq(k_t, wk_t, bk_all, 1, 0),
            vch(11), vch(12), vch(13), vch(14), vch(15),
            kq(k_t, wk_t, bk_all, 1, 1),
            kq(k_t, wk_t, bk_all, 1, 2),
            kq(k_t, wk_t, bk_all, 1, 3),
        ]
        for qh in range(1, NQH):
            for pair in range(NPAIR):
                feeder.append(kq(q_t, wq_t, bq_all, pair, qh))
        fi = [0]
        o_queue = []
        oi = [0]

        def feed(n):
            for _ in range(n):
                if fi[0] < len(feeder):
                    feeder[fi[0]]()
                    fi[0] += 1

        def feed_o(n):
            for _ in range(n):
                if oi[0] < len(o_queue):
                    o_queue[oi[0]]()
                    oi[0] += 1

        # ---- ramp: minimum to start slot 0 ----
        kq_chunk(k_t, wk_t, bk_all, 0, 0)
        kq_chunk(q_t, wq_t, bq_all, 0, 0)

        # ---- main loop: 8 groups x 16 slots ----
        groups = [(qh, pair) for qh in range(NQH) for pair in range(NPAIR)]
        Z_LAG = 4
        NG = len(groups)
        for gi, (qh, pair) in enumerate(groups):
            for t in range(NKB):
                s_slot(qh, pair, t, gi * NKB + t)
                if t >= Z_LAG:
                    z_group(qh, pair, t - Z_LAG)
                if gi > 0 and t < Z_LAG:
                    pqh, ppair = groups[gi - 1]
                    z_group(pqh, ppair, NKB - Z_LAG + t)
                    if t == Z_LAG - 1:
                        norm(pqh, ppair)
                        if ppair == NPAIR - 1:
                            # schedule output chunks for the finished q-block
                            for qc in range(4):
                                o_queue.append(
                                    (lambda a, b: (lambda: o_chunk(a, b)))
                                    (pqh, qc))
                # group 0 slots 0-3 are z-free: extra feed capacity there
                feed(2 if (gi == 0 and t < 4) else 1)
                # output chunks only after the norm chain has fully drained
                if t in (10, 12, 14):
                    feed_o(1)

        # ---- drain: last group's tail z, norm, and final output chunks ----
        lqh, lpair = groups[-1]
        for kb in range(NKB - Z_LAG, NKB):
            z_group(lqh, lpair, kb)
        norm(lqh, lpair)
        feed(len(feeder))
        feed_o(len(o_queue))
        for qc in range(4):
            o_chunk(lqh, qc)

        if dbg:
            nc.sync.dma_start(dbg_out["KT0"][:], k_t[0][:])
            nc.sync.dma_start(dbg_out["QT0"][:], q_t[0][:])
            nc.sync.dma_start(dbg_out["VA0"][:], v_aug[0][:])
            nc.sync.dma_start(dbg_out["ZT0"][:], z_t[0][:])
            nc.sync.dma_start(dbg_out["ZT1"][:], z_t[1][:])

    nc.compile()
    return nc


_NC = None
LAST_RESULTS = None


def _get_nc():
    global _NC
    if _NC is None:
        _NC = build_program(dbg=bool(int(os.environ.get("KERNEL_DEBUG", "0"))))
    return _NC


def _bf(x):
    return np.ascontiguousarray(np.asarray(x).astype(ml_dtypes.bfloat16))


def kernel(Q, K, V, Wq, bq, Wk, bk, Wv, bv, Wo, bo):
    global LAST_RESULTS
    nc = _get_nc()
    Qb, Kb, Vb = _bf(Q), _bf(K), _bf(V)
    Wqb, Wkb, Wvb, Wob = _bf(Wq), _bf(Wk), _bf(Wv), _bf(Wo)
    bqf = np.asarray(bq, np.float32)
    bkf = np.asarray(bk, np.float32)
    bvf = np.asarray(bv, np.float32)
    bof = np.asarray(bo, np.float32)
    in_maps = []
    for c in range(N_CORES):
        b, hg = c // 2, c % 2
        fs = slice(DG * hg, DG * (hg + 1))
        in_maps.append({
            "XQ": Qb[b],
            "XK": Kb[b],
            "XV": Vb[b],
            "Wq": np.ascontiguousarray(Wqb[:, fs]),
            "Wk": np.ascontiguousarray(Wkb[:, fs]),
            "Wv": np.ascontiguousarray(Wvb[:, fs]),
            "Wo": np.ascontiguousarray(Wob[fs, :]),
            "bq": np.ascontiguousarray(bqf[fs].reshape(DG, 1)),
            "bk": np.ascontiguousarray(bkf[fs].reshape(DG, 1)),
            "bv": np.ascontiguousarray(bvf[fs].reshape(1, DG)),
            "bo": np.ascontiguousarray((bof * 0.5).reshape(1, D)),
        })
    trace = bool(int(os.environ.get("KERNEL_TRACE", "0")))
    res = bass_utils.run_bass_kernel_spmd(
        nc, in_maps, core_ids=list(range(N_CORES)), trace=trace,
    )
    LAST_RESULTS = res
    out = np.empty((B, S, D), dtype=np.float32)
    for b in range(B):
        out[b] = res.results[2 * b]["OUT"] + res.results[2 * b + 1]["OUT"]
    return out


# revision 27
# speedup vs baseline: 1.2896x; 1.1242x over previous
"""Trainium2 Bass kernel for nn_MultiHeadAttention (B=4, S=2048, D=512, H=8, DH=64).

Sharding: 8 cores = 4 batches x 2 head-groups (tensor parallel over heads).
Each core projects Q/K/V for its 4 heads over the full 2048 rows (no
duplicated projection work), runs attention for those heads, and computes a
partial output projection (contraction over its 256 z-features). The host
sums the two partial outputs per batch.

Per-core pipeline (bf16 datapath, fp32 PSUM accumulation), organized as 128
uniform slots = 8 groups (4 q-blocks x 2 head-pairs) x 16 key-blocks:
  slot: S^T[k,1024] = [KA^T|KB^T](stationary) @ Q^T  (two 512-col matmuls,
        heads A/B side by side in one 2-bank PSUM tile, double-buffered)
        -> one ScalarE exp per slot ([128,1024] PSUM -> SBUF bf16)
        -> z^T[65,512] += [V_h|1](stationary) @ P^T   (lagged 4 slots)
  ScalarE does nothing but exp (the binding engine, ~131us); projections ride
  the PE slack as 512-col chunks through 2 spare PSUM banks; all bias adds,
  norm copies/mults on DVE; partition broadcasts on GpSimd.
"""

import os
import sys

import numpy as np

sys.path.insert(0, "/opt/trn_rl_repo")

import ml_dtypes
import concourse.bacc as bacc
import concourse.bass as bass
import concourse.mybir as mybir
import concourse.tile as tile
from concourse import bass_utils

F32 = mybir.dt.float32
BF16 = mybir.dt.bfloat16

B, S, D, H, DH = 4, 2048, 512, 8, 64
HG = H // 2          # heads per core (head-group)
DG = HG * DH         # 256 features per core
NKB = S // 128       # 16 k-blocks
NMC = D // 128       # 4 input-feature chunks
NQH = S // 512       # 4 query blocks of 512
NPAIR = HG // 2      # 2 head pairs per core
N_CORES = 8

Exp = mybir.ActivationFunctionType.Exp


def build_program(dbg=False):
    nc = bacc.Bacc("TRN2", target_bir_lowering=False, debug=False)
    dbg_out = {}
    if dbg:
        for nm, shp, dt in [("KT0", [128, S], BF16), ("QT0", [128, S], BF16),
                            ("VA0", [128, 260], BF16), ("P000", [128, 1024], BF16),
                            ("ZT0", [128, S], BF16), ("ZT1", [128, S], BF16),
                            ("ZC000", [65, 512], F32), ("RC000", [1, 512], F32),
                            ("ZC001", [65, 512], F32), ("RC001", [1, 512], F32)]:
            dbg_out[nm] = nc.dram_tensor(nm, shp, dt, kind="ExternalOutput").ap()

    # feature-major X^T, pre-transposed on the host (layout prep only)
    xq = nc.dram_tensor("XQT", [D, S], BF16, kind="ExternalInput").ap()
    xk = nc.dram_tensor("XKT", [D, S], BF16, kind="ExternalInput").ap()
    xv = nc.dram_tensor("XVT", [D, S], BF16, kind="ExternalInput").ap()
    wq = nc.dram_tensor("Wq", [D, DG], BF16, kind="ExternalInput").ap()
    wk = nc.dram_tensor("Wk", [D, DG], BF16, kind="ExternalInput").ap()
    wv = nc.dram_tensor("Wv", [D, DG], BF16, kind="ExternalInput").ap()
    wo = nc.dram_tensor("Wo", [DG, D], BF16, kind="ExternalInput").ap()
    bq = nc.dram_tensor("bq", [DG, 1], F32, kind="ExternalInput").ap()
    bk = nc.dram_tensor("bk", [DG, 1], F32, kind="ExternalInput").ap()
    bv = nc.dram_tensor("bv", [1, DG], F32, kind="ExternalInput").ap()
    bo = nc.dram_tensor("bo", [1, D], F32, kind="ExternalInput").ap()
    out = nc.dram_tensor("OUT", [S, D], F32, kind="ExternalOutput").ap()

    from contextlib import ExitStack

    with tile.TileContext(nc) as tc, ExitStack() as ctx:
        const = ctx.enter_context(tc.tile_pool(name="const", bufs=1))
        xt_pool = ctx.enter_context(tc.tile_pool(name="xt", bufs=1))
        w_pool = ctx.enter_context(tc.tile_pool(name="w", bufs=1))
        kt_pool = ctx.enter_context(tc.tile_pool(name="kt", bufs=1))
        qt_pool = ctx.enter_context(tc.tile_pool(name="qt", bufs=1))
        v_pool = ctx.enter_context(tc.tile_pool(name="v", bufs=1))
        p_pool = ctx.enter_context(tc.tile_pool(name="p", bufs=6))
        zt_pool = ctx.enter_context(tc.tile_pool(name="zt", bufs=1))
        nrm_pool = ctx.enter_context(tc.tile_pool(name="nrm", bufs=3))
        out_pool = ctx.enter_context(tc.tile_pool(name="outp", bufs=2))

        # PSUM: s0/s1 [128,1024] (2 banks each), zA/zB [65->128,512] (1 bank
        # each), pr0/pr1 [128,512] (1 bank each) = 8 banks exactly.
        ps = ctx.enter_context(tc.tile_pool(name="ps", bufs=1, space="PSUM"))
        pr_ctr = [0]

        def pr_tile(cols, name):
            tag = f"pr{pr_ctr[0] % 2}"
            pr_ctr[0] += 1
            return ps.tile([128, cols], F32, tag=tag, name=name,
                           padded_shape=[128, 512])

        # ---- warm the ScalarE Exp table immediately ----
        warm = nrm_pool.tile([1, 8], F32, tag="warm")
        nc.gpsimd.memset(warm[:], 0.0)
        warm2 = nrm_pool.tile([1, 8], F32, tag="warm2")
        nc.scalar.activation(warm2[:], warm[:], Exp, scale=0.125)

        # ---- weight loads: one DMA per tensor (sync queue) ----
        def load_w(wdram, name, cols, nch=NMC, eng=None):
            big = w_pool.tile([128, nch * cols], BF16, tag=f"w{name}",
                              name=f"w{name}")
            (eng or nc.sync).dma_start(
                big[:].rearrange("p (g c) -> p g c", g=nch),
                wdram.rearrange("(g p) c -> p g c", p=128),
            )
            return [big[:, cols * mc:cols * (mc + 1)] for mc in range(nch)]

        # ---- X^T loads: one plain rearranged DMA per tensor (the host
        # pre-transposed; DMA transposes cost ~1.5us serial issue each and
        # corrupt when issued concurrently from two HWDGE queues) ----
        def load_xt(xdram, name):
            big = xt_pool.tile([128, NMC * S], BF16, tag=f"xt{name}",
                               name=f"xt{name}")
            nc.sync.dma_start(
                big[:].rearrange("p (g c) -> p g c", g=NMC),
                xdram.rearrange("(g p) c -> p g c", p=128),
            )
            return [big[:, S * mc:S * (mc + 1)] for mc in range(NMC)]

        # tiny bias DMAs first so the first bias-adds are never blocked
        bq_all = const.tile([128, NPAIR], F32, tag="bqa")
        nc.sync.dma_start(
            bq_all[:].rearrange("p (g o) -> p g o", g=NPAIR),
            bq.rearrange("(g p) o -> p g o", p=128),
        )
        bk_all = const.tile([128, NPAIR], F32, tag="bka")
        nc.sync.dma_start(
            bk_all[:].rearrange("p (g o) -> p g o", g=NPAIR),
            bk.rearrange("(g p) o -> p g o", p=128),
        )
        bv_row = const.tile([1, DG], F32, tag="bvr")
        nc.sync.dma_start(bv_row[:], bv[:])
        bo_row = const.tile([1, D], F32, tag="bor")
        nc.sync.dma_start(bo_row[:], bo[:])

        # all input DMAs on the sync HWDGE queue, K before Q before V so the
        # pipeline's first consumers unblock earliest
        wk_t = load_w(wk, "k", DG)
        xkt = load_xt(xk, "k")
        wq_t = load_w(wq, "q", DG)
        xqt = load_xt(xq, "q")
        wv_t = load_w(wv, "v", DG)
        xvt = load_xt(xv, "v")
        wo_t = load_w(wo, "o", D, nch=NPAIR)

        bv_bc = const.tile([128, DG], F32, tag="bvb")
        nc.gpsimd.partition_broadcast(bv_bc[:], bv_row[:], channels=128)
        bo_bc = const.tile([128, D], F32, tag="bob")
        nc.gpsimd.partition_broadcast(bo_bc[:], bo_row[:], channels=128)

        # ---- persistent SBUF results ----
        k_t = [kt_pool.tile([128, S], BF16, tag=f"kt{p}", name=f"kt{p}")
               for p in range(NPAIR)]
        q_t = [qt_pool.tile([128, S], BF16, tag=f"qt{p}", name=f"qt{p}")
               for p in range(NPAIR)]
        VW = HG * (DH + 1)  # 260: per head 64 value cols + 1 ones col
        v_aug = [v_pool.tile([128, VW], BF16, tag=f"v{kb}", name=f"v{kb}")
                 for kb in range(NKB)]
        z_t = [zt_pool.tile([128, S], BF16, tag=f"zt{p}", name=f"zt{p}")
               for p in range(NPAIR)]

        # ---- projection chunks (feeder work) ----
        def kq_chunk(dst, w_ts, b_all, pair, cb):
            pj = pr_tile(512, f"pj{cb}")
            for mc in range(NMC):
                nc.tensor.matmul(
                    pj[:],
                    w_ts[mc][:, 128 * pair:128 * (pair + 1)],
                    (xkt if dst is k_t else xqt)[mc][:, 512 * cb:512 * (cb + 1)],
                    start=(mc == 0),
                    stop=(mc == NMC - 1),
                )
            nc.vector.tensor_scalar_add(
                dst[pair][:, 512 * cb:512 * (cb + 1)], pj[:],
                b_all[:, pair:pair + 1],
            )

        def v_chunk(kb):
            nc.gpsimd.memset(
                v_aug[kb][:].rearrange("p (h c) -> p h c", h=HG)[:, :, DH:DH + 1],
                1.0,
            )
            pj = pr_tile(DG, f"pjv{kb}")
            for mc in range(NMC):
                nc.tensor.matmul(
                    pj[:],
                    xvt[mc][:, 128 * kb:128 * (kb + 1)],
                    wv_t[mc][:],
                    start=(mc == 0),
                    stop=(mc == NMC - 1),
                )
            nc.vector.tensor_add(
                v_aug[kb][:].rearrange("p (h c) -> p h c", h=HG)[:, :, 0:DH],
                pj[:].rearrange("p (h c) -> p h c", h=HG),
                bv_bc[:].rearrange("p (h c) -> p h c", h=HG),
            )

        def o_chunk(qh, qc):
            po = pr_tile(512, f"po{qh}{qc}")
            qs = slice(512 * qh + 128 * qc, 512 * qh + 128 * (qc + 1))
            for p2 in range(NPAIR):
                nc.tensor.matmul(
                    po[:],
                    z_t[p2][:, qs],
                    wo_t[p2][:],
                    start=(p2 == 0),
                    stop=(p2 == NPAIR - 1),
                )
            ot = out_pool.tile([128, D], F32, tag="ot")
            nc.vector.tensor_add(ot[:], po[:], bo_bc[:])
            nc.sync.dma_start(out[qs, :], ot[:])

        # ---- attention slot machinery ----
        p_slabs = {}
        z_tiles = [None]

        def s_slot(qh, pair, kb, sl):
            st = ps.tile([128, 1024], F32, tag=f"s{sl % 2}",
                         name=f"s{qh}_{pair}_{kb}", padded_shape=[128, 1024])
            qs = slice(512 * qh, 512 * (qh + 1))
            ks = slice(128 * kb, 128 * (kb + 1))
            nc.tensor.matmul(st[:, 0:512], k_t[pair][0:DH, ks],
                             q_t[pair][0:DH, qs],
                             start=True, stop=True, tile_position=(0, 0))
            nc.tensor.matmul(st[:, 512:1024], k_t[pair][DH:128, ks],
                             q_t[pair][DH:128, qs],
                             start=True, stop=True, tile_position=(64, 0))
            pab = p_pool.tile([128, 1024], BF16, tag="p",
                              name=f"p{qh}_{pair}_{kb}")
            nc.scalar.activation(pab[:], st[:], Exp, scale=0.125)
            if dbg and (qh, pair, kb) == (0, 0, 0):
                nc.sync.dma_start(dbg_out["P000"][:], pab[:])
            p_slabs[(qh, pair, kb)] = pab

        def z_alloc():
            zA = ps.tile([DH + 1, 512], F32, tag="zA", name="zA",
                         padded_shape=[128, 512])
            zB = ps.tile([DH + 1, 512], F32, tag="zB", name="zB",
                         padded_shape=[128, 512])
            z_tiles[0] = (zA, zB)

        def z_group(qh, pair, kb):
            if kb == 0:
                z_alloc()
            zA, zB = z_tiles[0]
            pab = p_slabs.pop((qh, pair, kb))
            hA, hB = 2 * pair, 2 * pair + 1
            nc.tensor.matmul(zA[:], v_aug[kb][:, 65 * hA:65 * hA + 65],
                             pab[:, 0:512],
                             start=(kb == 0), stop=(kb == NKB - 1),
                             skip_group_check=True)
            nc.tensor.matmul(zB[:], v_aug[kb][:, 65 * hB:65 * hB + 65],
                             pab[:, 512:1024],
                             start=(kb == 0), stop=(kb == NKB - 1),
                             skip_group_check=True)

        def norm(qh, pair):
            zA, zB = z_tiles[0]
            zcs = []
            for z_ps, half in ((zA, 0), (zB, 1)):
                # copy PSUM->SBUF first so both z banks free up fast
                zc = nrm_pool.tile([DH + 1, 512], F32, tag=f"zc{half}")
                nc.vector.tensor_copy(zc[:], z_ps[:])
                zcs.append(zc)
            for zc, half in zip(zcs, (0, 1)):
                # custom-DVE recip needs a fresh partition-0 source tile
                row = nrm_pool.tile([1, 512], F32, tag="row")
                nc.vector.tensor_copy(row[:], zc[DH:DH + 1, :])
                recip = nrm_pool.tile([1, 512], F32, tag="recip")
                nc.vector.reciprocal_approx_fast(recip[:], row[:])
                rbc = nrm_pool.tile([DH, 512], F32, tag="rbc")
                nc.gpsimd.partition_broadcast(rbc[:], recip[:], channels=DH)
                nc.vector.tensor_mul(
                    z_t[pair][64 * half:64 * half + 64,
                              512 * qh:512 * (qh + 1)],
                    zc[0:DH, :], rbc[:],
                )
                if dbg and (qh, pair) == (0, 0):
                    nc.sync.dma_start(dbg_out[f"ZC00{half}"][:], zc[:])
                    nc.sync.dma_start(dbg_out[f"RC00{half}"][:], recip[:])

        # ---- feeder schedule (ordered by first-use time with margin) ----
        def kq(dst, w_ts, b_all, pair, cb):
            return lambda: kq_chunk(dst, w_ts, b_all, pair, cb)

        def vch(kb):
            return lambda: v_chunk(kb)

        # ordered by first-use slot; group 0 consumes 2/slot in slots 0-3,
        # then 1/slot (see feed calls below)
        feeder = [
            kq(k_t, wk_t, bk_all, 0, 1), vch(0),
            kq(q_t, wq_t, bq_all, 1, 0), vch(1),
            kq(k_t, wk_t, bk_all, 0, 2), vch(2),
            kq(k_t, wk_t, bk_all, 0, 3), vch(3),
            vch(4), vch(5), vch(6), vch(7), vch(8), vch(9), vch(10),
            kq(k_t, wk_t, bk_all, 1, 0),
            vch(11), vch(12), vch(13), vch(14), vch(15),
            kq(k_t, wk_t, bk_all, 1, 1),
            kq(k_t, wk_t, bk_all, 1, 2),
            kq(k_t, wk_t, bk_all, 1, 3),
        ]
        for qh in range(1, NQH):
            for pair in range(NPAIR):
                feeder.append(kq(q_t, wq_t, bq_all, pair, qh))
        fi = [0]
        o_queue = []
        oi = [0]

        def feed(n):
            for _ in range(n):
                if fi[0] < len(feeder):
                    feeder[fi[0]]()
                    fi[0] += 1

        def feed_o(n):
            for _ in range(n):
                if oi[0] < len(o_queue):
                    o_queue[oi[0]]()
                    oi[0] += 1

        # ---- ramp: minimum to start slot 0 ----
        kq_chunk(k_t, wk_t, bk_all, 0, 0)
        kq_chunk(q_t, wq_t, bq_all, 0, 0)

        # ---- main loop: 8 groups x 16 slots ----
        groups = [(qh, pair) for qh in range(NQH) for pair in range(NPAIR)]
        Z_LAG = 4
        NG = len(groups)
        for gi, (qh, pair) in enumerate(groups):
            for t in range(NKB):
                s_slot(qh, pair, t, gi * NKB + t)
                if t >= Z_LAG:
                    z_group(qh, pair, t - Z_LAG)
                if gi > 0 and t < Z_LAG:
                    pqh, ppair = groups[gi - 1]
                    z_group(pqh, ppair, NKB - Z_LAG + t)
                    if t == Z_LAG - 1:
                        norm(pqh, ppair)
                        if ppair == NPAIR - 1:
                            # schedule output chunks for the finished q-block
                            for qc in range(4):
                                o_queue.append(
                                    (lambda a, b: (lambda: o_chunk(a, b)))
                                    (pqh, qc))
                # group 0 slots 0-3 are z-free: extra feed capacity there
                feed(2 if (gi == 0 and t < 4) else 1)
                # output chunks only after the norm chain has fully drained
                if t in (10, 12, 14):
                    feed_o(1)

        # ---- drain: last group's tail z, norm, and final output chunks ----
        lqh, lpair = groups[-1]
        for kb in range(NKB - Z_LAG, NKB):
            z_group(lqh, lpair, kb)
        norm(lqh, lpair)
        feed(len(feeder))
        feed_o(len(o_queue))
        for qc in range(4):
            o_chunk(lqh, qc)

        if dbg:
            nc.sync.dma_start(dbg_out["KT0"][:], k_t[0][:])
            nc.sync.dma_start(dbg_out["QT0"][:], q_t[0][:])
            nc.sync.dma_start(dbg_out["VA0"][:], v_aug[0][:])
            nc.sync.dma_start(dbg_out["ZT0"][:], z_t[0][:])
            nc.sync.dma_start(dbg_out["ZT1"][:], z_t[1][:])

    nc.compile()
    return nc


_NC = None
LAST_RESULTS = None


def _get_nc():
    global _NC
    if _NC is None:
        _NC = build_program(dbg=bool(int(os.environ.get("KERNEL_DEBUG", "0"))))
    return _NC


def _bf(x):
    return np.ascontiguousarray(np.asarray(x).astype(ml_dtypes.bfloat16))


def kernel(Q, K, V, Wq, bq, Wk, bk, Wv, bv, Wo, bo):
    global LAST_RESULTS
    nc = _get_nc()
    Qb, Kb, Vb = _bf(Q), _bf(K), _bf(V)
    Wqb, Wkb, Wvb, Wob = _bf(Wq), _bf(Wk), _bf(Wv), _bf(Wo)
    bqf = np.asarray(bq, np.float32)
    bkf = np.asarray(bk, np.float32)
    bvf = np.asarray(bv, np.float32)
    bof = np.asarray(bo, np.float32)
    QbT = np.ascontiguousarray(Qb.transpose(0, 2, 1))
    KbT = np.ascontiguousarray(Kb.transpose(0, 2, 1))
    VbT = np.ascontiguousarray(Vb.transpose(0, 2, 1))
    in_maps = []
    for c in range(N_CORES):
        b, hg = c // 2, c % 2
        fs = slice(DG * hg, DG * (hg + 1))
        in_maps.append({
            "XQT": QbT[b],
            "XKT": KbT[b],
            "XVT": VbT[b],
            "Wq": np.ascontiguousarray(Wqb[:, fs]),
            "Wk": np.ascontiguousarray(Wkb[:, fs]),
            "Wv": np.ascontiguousarray(Wvb[:, fs]),
            "Wo": np.ascontiguousarray(Wob[fs, :]),
            "bq": np.ascontiguousarray(bqf[fs].reshape(DG, 1)),
            "bk": np.ascontiguousarray(bkf[fs].reshape(DG, 1)),
            "bv": np.ascontiguousarray(bvf[fs].reshape(1, DG)),
            "bo": np.ascontiguousarray((bof * 0.5).reshape(1, D)),
        })
    trace = bool(int(os.environ.get("KERNEL_TRACE", "0")))
    res = bass_utils.run_bass_kernel_spmd(
        nc, in_maps, core_ids=list(range(N_CORES)), trace=trace,
    )
    LAST_RESULTS = res
    out = np.empty((B, S, D), dtype=np.float32)
    for b in range(B):
        out[b] = res.results[2 * b]["OUT"] + res.results[2 * b + 1]["OUT"]
    return out
